# revision 28
# baseline (speedup 1.0000x reference)
"""Trainium2 Bass kernel for the CrossAttention (linear-attention style) module.

Math (per batch b, head h, stream s in {rgb, x}):
    K = A_s @ Wk_s^T, V = A_s @ Wv_s^T            (A_s = stream input [N, C])
    ctx_s = softmax(scale * K^T V, axis=rows)     # [32, 32] per head
    out_s = A_s @ blockdiag(ctx_{s'})             # s' = the OTHER stream

Key identity: K^T V = Wk (A^T A) Wv^T, so the big inputs only feed the Gram
matrix G = A^T A (one [256,256] per (batch, stream)); the rest is tiny.

Sharding: 8 cores = 4 batches x 2 streams.  Core 2b+s owns stream s of
batch b: it streams A_s once (fp16, partition-major layout prepared on
host), computes G via PSUM-accumulated matmuls, derives its own ctx_s
on-chip, then swaps ctx with its pair core through a tiny AllReduce
(peer = sum - own, so the SPMD program stays core-independent).  While the
collective is in flight the PE transposes the retained input tiles
(plain matmuls against identity - NOT transpose-mode, which is slow).
Finally out^T = blockdiag(ctx_peer) block-stationary @ A^T in just
64 N=512 matmuls, written fp16 transposed; the host untransposes.

Hardware notes baked in:
 - Each Gram accumulation region gets its OWN PSUM bank: a start=True
   matmul clears has_written BANK-WIDE, so interleaving two accumulation
   groups in one bank corrupts the other region's in-flight tile
   (measured: split banks are exact to 1e-3 absolute; shared bank loses
   ~half a tile per group start).
 - PSUM->SBUF cast copies run ~1 elem/cycle/lane on both DVE and ACT,
   ~780 ns per [128,512]; they are the real phase gate, so they alternate
   between the two engines everywhere.

Everything on the streamed path is fp16: PE runs at full rate and DMA
traffic halves vs fp32 (8 MB in + 8 MB out per core).  Verified by a host
simulation of the exact pipeline: rel err ~5.6e-3 (gate is 2e-2).
"""

import sys

if "/opt/trn_rl_repo" not in sys.path:
    sys.path.insert(0, "/opt/trn_rl_repo")

import numpy as np

import concourse.bass as bass
import concourse.mybir as mybir
import concourse.tile as tile
from concourse import bacc
from concourse.bass import ds, ts
from concourse.bass_utils import run_bass_kernel_spmd
from concourse.tile_rust import add_dep_helper

P = 128
C = 256
HD = 32
SCALE = HD ** -0.5
F16 = mybir.dt.float16
F32 = mybir.dt.float32

B_FULL = 4
N_FULL = 16384
H_FULL = 8

N_TILES = N_FULL // P          # 128
TPC = 16                       # tiles per chunk
N_CHUNKS = N_TILES // TPC      # 8
SPAN = 4                       # qT tiles per out-matmul -> N = 512

REPLICA_GROUPS = [[0, 1], [2, 3], [4, 5], [6, 7]]


def build_module(num_devices=8):
    nc = bacc.Bacc(
        "TRN2",
        target_bir_lowering=False,
        debug=False,
        enable_asserts=False,
        num_devices=num_devices,
    )
    a_pm = nc.dram_tensor("a_pm", [P, N_TILES, C], F16, kind="ExternalInput").ap()
    wkT = nc.dram_tensor("wkT", [P, 2, C], F16, kind="ExternalInput").ap()
    wvT = nc.dram_tensor("wvT", [P, 2, C], F16, kind="ExternalInput").ap()
    ident_d = nc.dram_tensor("ident", [P, P], F16, kind="ExternalInput").ap()
    oT = nc.dram_tensor("oT", [P, 2, N_FULL], F16, kind="ExternalOutput").ap()

    with tile.TileContext(nc) as tc:
        _build_kernel(tc, a_pm, wkT, wvT, ident_d, oT)
    nc.compile()
    return nc


def _build_kernel(tc, a_pm, wkT_d, wvT_d, ident_d, oT):
    nc = tc.nc

    with (
        tc.tile_pool(name="persist", bufs=1) as persist,
        tc.tile_pool(name="stage", bufs=3) as stage,
        tc.tile_pool(name="dram", bufs=1, space="DRAM") as dram,
        tc.tile_pool(name="psum_ga", bufs=1, space="PSUM") as psum_ga,
        tc.tile_pool(name="psum_gb", bufs=1, space="PSUM") as psum_gb,
        tc.tile_pool(name="psum_l", bufs=1, space="PSUM") as psum_l,
        tc.tile_pool(name="psum_big", bufs=5, space="PSUM") as psum_big,
    ):
        # ---- persistent SBUF state ----
        in_sb = [
            persist.tile([P, TPC, C], F16, tag=f"in{ch}", name=f"in{ch}")
            for ch in range(N_CHUNKS)
        ]
        qT_sb = [
            persist.tile([P, 2, TPC, P], F16, tag=f"qT{ch}", name=f"qT{ch}")
            for ch in range(N_CHUNKS)
        ]
        w_k = persist.tile([P, 2, C], F16, tag="w_k")
        w_v = persist.tile([P, 2, C], F16, tag="w_v")
        ident = persist.tile([P, P], F16, tag="ident")
        g16 = persist.tile([P, 2, C], F16, tag="g16")
        T16 = persist.tile([P, 2, C], F16, tag="T16")
        lgc = persist.tile([P, 2, HD], F32, tag="lgc")    # compact logits^T
        cT_own = persist.tile([P, 2, HD], F32, tag="cT_own")
        cT_sum = persist.tile([P, 2, HD], F32, tag="cT_sum")
        cT_peer = persist.tile([P, 2, HD], F32, tag="cT_peer")
        ctx16 = persist.tile([P, 2, P], F16, tag="ctx16")  # blockdiag, fp16

        b_in = dram.tile([P, 2, HD], F32, tag="b_in", name="b_in")
        b_out = dram.tile([P, 2, HD], F32, tag="b_out", name="b_out")

        # ---- input stream first (chunk 0 split for an earlier Gram start) ----
        nc.sync.dma_start(in_sb[0][:, ts(0, TPC // 2), :], a_pm[:, ts(0, TPC // 2), :])
        nc.sync.dma_start(
            in_sb[0][:, ds(TPC // 2, TPC // 2), :], a_pm[:, ds(TPC // 2, TPC // 2), :]
        )
        for ch in range(1, N_CHUNKS):
            nc.sync.dma_start(in_sb[ch][:], a_pm[:, ts(ch, TPC), :])
        nc.sync.dma_start(w_k[:], wkT_d)
        nc.sync.dma_start(w_v[:], wvT_d)
        nc.sync.dma_start(ident[:], ident_d)
        nc.vector.memset(ctx16[:], 0.0)

        # ---- phase 1: Gram G = A^T A; one accumulation region PER BANK ----
        ctx_last = {}  # engine -> last ctx-chain instruction (ordering fences)
        copy_first = {}

        def transpose_pair(ch, tp):
            """PE-transpose tiles (2*tp, 2*tp+1) of chunk ch into qT_sb."""
            pt = psum_big.tile([P, 2, 2, P], F32, tag="big", name=f"pt{ch}_{tp}")
            for i in range(2):
                for t2 in range(2):
                    nc.tensor.matmul(
                        pt[:, i, t2, :],
                        in_sb[ch][:, tp * 2 + t2, ts(i, P)],
                        ident[:], start=True, stop=True,
                    )
            dst = qT_sb[ch][:, :, ts(tp, 2), :]
            # same destination tile -> keep one engine per chunk (avoids
            # cross-engine write serialization), alternate by chunk parity
            eng = "v" if ch % 2 == 0 else "s"
            if eng == "v":
                cp = nc.vector.tensor_copy(dst, pt[:])
            else:
                cp = nc.scalar.copy(dst, pt[:])
            # keep the ctx->collective critical chain ahead of the copy
            # flood in each engine's in-order queue
            if eng not in copy_first:
                copy_first[eng] = True
                if eng in ctx_last:
                    add_dep_helper(
                        cp.ins, ctx_last[eng].ins, sync=False,
                        reason="qT copies wait for ctx chain",
                    )

        pga = psum_ga.tile([P, 2, C], F32, tag="ga", name="pga")  # G[0:128, :]
        pgb = psum_gb.tile([P, 2, C], F32, tag="gb", name="pgb")  # G[128:256, :]
        for ch in range(N_CHUNKS):
            for t in range(TPC):
                ti = ch * TPC + t
                tile_ap = in_sb[ch][:, t, :]
                nc.tensor.matmul(
                    pga[:, 0, :], tile_ap[:, ts(0, P)], tile_ap,
                    start=(ti == 0), stop=(ti == N_TILES - 1),
                )
                nc.tensor.matmul(
                    pgb[:, 0, :], tile_ap[:, ts(1, P)], tile_ap,
                    start=(ti == 0), stop=(ti == N_TILES - 1),
                )
        nc.vector.tensor_copy(g16[:, 0, :], pga[:, 0, :])
        nc.scalar.copy(g16[:, 1, :], pgb[:, 0, :])

        # ---- ctx_own: T = G @ Wk^T (G symmetric), logits = Wv @ T ----
        # pT blocks reuse the two Gram banks (same tag ring -> sequenced).
        pTA = psum_ga.tile([P, 2, C], F32, tag="ga", name="pTA")  # T[0:128, :]
        pTB = psum_gb.tile([P, 2, C], F32, tag="gb", name="pTB")  # T[128:256, :]
        for blkc, pT in ((0, pTA), (1, pTB)):
            for ci in range(2):
                nc.tensor.matmul(
                    pT[:, 0, :], g16[:, ci, ts(blkc, P)], w_k[:, ci, :],
                    start=(ci == 0), stop=(ci == 1),
                )
        nc.vector.tensor_copy(T16[:, 0, :], pTA[:, 0, :])
        nc.scalar.copy(T16[:, 1, :], pTB[:, 0, :])

        for g in range(2):
            pl = psum_l.tile([P, P], F32, tag="pl", name=f"pl{g}")
            for ci in range(2):
                nc.tensor.matmul(
                    pl[:], w_v[:, ci, ts(g, P)], T16[:, ci, ts(g, P)],
                    start=(ci == 0), stop=(ci == 1),
                )
            # extract the 4 diagonal head blocks -> compact [128, 32]
            for h in range(4):
                nc.vector.tensor_copy(
                    lgc[ds(h * HD, HD), g, :], pl[ds(h * HD, HD), ds(h * HD, HD)]
                )
            # batched softmax over d (free axis) for all 4 heads at once
            mx = stage.tile([P, 1], F32, tag="mx", name=f"mx{g}")
            nc.vector.tensor_reduce(
                mx[:], lgc[:, g, :], axis=mybir.AxisListType.X, op=mybir.AluOpType.max
            )
            nmx = stage.tile([P, 1], F32, tag="nmx", name=f"nmx{g}")
            nc.vector.tensor_scalar_mul(nmx[:], mx[:], -SCALE)
            sm = stage.tile([P, HD], F32, tag="sm", name=f"sm{g}")
            ssum = stage.tile([P, 1], F32, tag="ssum", name=f"ssum{g}")
            ctx_last["s"] = nc.scalar.activation(
                sm[:], lgc[:, g, :], mybir.ActivationFunctionType.Exp,
                bias=nmx[:], scale=SCALE, accum_out=ssum[:],
            )
            rs = stage.tile([P, 1], F32, tag="rs", name=f"rs{g}")
            nc.vector.reciprocal(rs[:], ssum[:])
            smn = stage.tile([P, HD], F32, tag="smn", name=f"smn{g}")
            nc.vector.tensor_scalar_mul(smn[:], sm[:], rs[:])
            # per-head 32x32 transpose: [32h+e, d] -> [32h+d, e]
            ctx_last["v"] = nc.vector.transpose(cT_own[:, g, :], smn[:])

        # ---- exchange ctx with the pair core (peer = pairsum - own).
        # Bounce DMAs ride the idle sync queue: gpsimd's wait lanes get
        # polluted by the transpose-phase copies and fire ~13us late.
        nc.sync.dma_start(b_in[:], cT_own[:])
        nc.gpsimd.collective_compute(
            "AllReduce",
            mybir.AluOpType.add,
            replica_groups=REPLICA_GROUPS,
            ins=[b_in.opt()],
            outs=[b_out.opt()],
        )
        nc.sync.dma_start(cT_sum[:], b_out[:])

        # ---- overlap the collective (~26us): PE transposes the input ----
        for ch in range(N_CHUNKS):
            for tp in range(TPC // 2):
                transpose_pair(ch, tp)

        nc.vector.tensor_sub(cT_peer[:], cT_sum[:], cT_own[:])
        # scatter-cast peer ctx into fp16 blockdiag form
        for g in range(2):
            for h in range(4):
                nc.vector.tensor_copy(
                    ctx16[ds(h * HD, HD), g, ds(h * HD, HD)],
                    cT_peer[ds(h * HD, HD), g, :],
                )

        # ---- out^T = ctx_blk (stationary) @ qT spans, fp16 staged, DMA out.
        # One staging tile + one DMA per matmul: the two copy engines and the
        # two DMA trigger queues (sync HWDGE, gpsimd SWDGE) run concurrently.
        for g in range(2):
            for q in range(N_TILES // SPAN):
                ch, sp = divmod(q, TPC // SPAN)
                po = psum_big.tile([P, SPAN * P], F32, tag="big", name=f"po{g}_{q}")
                nc.tensor.matmul(
                    po[:], ctx16[:, g, :], qT_sb[ch][:, g, ts(sp, SPAN), :],
                    start=True, stop=True,
                )
                st = stage.tile(
                    [P, SPAN * P], F16, tag="st", bufs=16, name=f"st{g}_{q}"
                )
                if q % 2 == 0:
                    nc.vector.tensor_copy(st[:], po[:])
                    nc.sync.dma_start(oT[:, g, ts(q, SPAN * P)], st[:])
                else:
                    nc.scalar.copy(st[:], po[:])
                    nc.gpsimd.dma_start(oT[:, g, ts(q, SPAN * P)], st[:])


# ---------------------------------------------------------------------------
# Host-side wrapper
# ---------------------------------------------------------------------------

_NC_CACHE = {}


def _get_module(**kw):
    key = tuple(sorted(kw.items()))
    if key not in _NC_CACHE:
        _NC_CACHE[key] = build_module(**kw)
    return _NC_CACHE[key]


def make_in_maps(rgb, x, Wkv_rgb, Wkv_x, n_cores=8):
    """Per-core inputs. Core 2b+s owns stream s (0=rgb, 1=x) of batch b."""
    eye = np.eye(P, dtype=np.float16)
    in_maps = []
    for core in range(n_cores):
        b, s = divmod(core, 2)
        A = (rgb if s == 0 else x)[b]
        W = Wkv_rgb if s == 0 else Wkv_x
        a16 = A.astype(np.float16)
        a_pm = np.ascontiguousarray(a16.reshape(N_TILES, P, C).transpose(1, 0, 2))
        WkT = W[:C].T.reshape(2, P, C).transpose(1, 0, 2)   # [p, ci, col]
        WvT = W[C:].T.reshape(2, P, C).transpose(1, 0, 2)
        in_maps.append(
            {
                "a_pm": a_pm,
                "wkT": np.ascontiguousarray(WkT.astype(np.float16)),
                "wvT": np.ascontiguousarray(WvT.astype(np.float16)),
                "ident": eye,
            }
        )
    return in_maps


def assemble(results):
    out_rgb = np.empty((B_FULL, N_FULL, C), dtype=np.float32)
    out_x = np.empty_like(out_rgb)
    for core, res in enumerate(results):
        b, s = divmod(core, 2)
        o = res["oT"].transpose(2, 1, 0).reshape(N_FULL, C).astype(np.float32)
        (out_rgb if s == 0 else out_x)[b] = o
    return out_rgb, out_x


def kernel(rgb, x, Wkv_rgb, Wkv_x, num_heads):
    rgb = np.asarray(rgb, dtype=np.float32)
    x = np.asarray(x, dtype=np.float32)
    Wkv_rgb = np.asarray(Wkv_rgb, dtype=np.float32)
    Wkv_x = np.asarray(Wkv_x, dtype=np.float32)
    assert int(num_heads) == H_FULL
    assert rgb.shape == (B_FULL, N_FULL, C) and x.shape == (B_FULL, N_FULL, C)

    nc = _get_module()
    in_maps = make_in_maps(rgb, x, Wkv_rgb, Wkv_x)
    res = run_bass_kernel_spmd(nc, in_maps, core_ids=list(range(8)))
    return assemble(res.results)


# revision 29
# speedup vs baseline: 1.0463x; 1.0463x over previous
"""Trainium2 Bass kernel for the CrossAttention (linear-attention style) module.

Math (per batch b, head h, stream s in {rgb, x}):
    K = A_s @ Wk_s^T, V = A_s @ Wv_s^T            (A_s = stream input [N, C])
    ctx_s = softmax(scale * K^T V, axis=rows)     # [32, 32] per head
    out_s = A_s @ blockdiag(ctx_{s'})             # s' = the OTHER stream

Key identity: K^T V = Wk (A^T A) Wv^T, so the big inputs only feed the Gram
matrix G = A^T A (one [256,256] per (batch, stream)); the rest is tiny.

Sharding: 8 cores = 4 batches x 2 streams.  Core 2b+s owns stream s of
batch b: it streams A_s once (fp16, partition-major layout prepared on
host), computes G via PSUM-accumulated matmuls, derives its own ctx_s
on-chip, then swaps ctx with its pair core through a tiny AllReduce
(peer = sum - own, so the SPMD program stays core-independent).  While the
collective is in flight the PE transposes the retained input tiles
(plain matmuls against identity - NOT transpose-mode, which is slow).
Finally out^T = blockdiag(ctx_peer) block-stationary @ A^T in just
64 N=512 matmuls, written fp16 transposed; the host untransposes.

Hardware notes baked in:
 - Each Gram accumulation region gets its OWN PSUM bank: a start=True
   matmul clears has_written BANK-WIDE, so interleaving two accumulation
   groups in one bank corrupts the other region's in-flight tile
   (measured: split banks are exact to 1e-3 absolute; shared bank loses
   ~half a tile per group start).
 - PSUM->SBUF cast copies run ~1 elem/cycle/lane on both DVE and ACT,
   ~780 ns per [128,512]; they are the real phase gate, so they alternate
   between the two engines everywhere.

Everything on the streamed path is fp16: PE runs at full rate and DMA
traffic halves vs fp32 (8 MB in + 8 MB out per core).  Verified by a host
simulation of the exact pipeline: rel err ~5.6e-3 (gate is 2e-2).
"""

import sys

if "/opt/trn_rl_repo" not in sys.path:
    sys.path.insert(0, "/opt/trn_rl_repo")

import numpy as np

import concourse.bass as bass
import concourse.mybir as mybir
import concourse.tile as tile
from concourse import bacc
from concourse.bass import ds, ts
from concourse.bass_utils import run_bass_kernel_spmd
from concourse.tile_rust import add_dep_helper

P = 128
C = 256
HD = 32
SCALE = HD ** -0.5
F16 = mybir.dt.float16
F32 = mybir.dt.float32

B_FULL = 4
N_FULL = 16384
H_FULL = 8

N_TILES = N_FULL // P          # 128
TPC = 16                       # tiles per chunk
N_CHUNKS = N_TILES // TPC      # 8
SPAN = 4                       # qT tiles per out-matmul -> N = 512

REPLICA_GROUPS = [[0, 1], [2, 3], [4, 5], [6, 7]]


def build_module(num_devices=8):
    nc = bacc.Bacc(
        "TRN2",
        target_bir_lowering=False,
        debug=False,
        enable_asserts=False,
        num_devices=num_devices,
    )
    a_pm = nc.dram_tensor("a_pm", [P, N_TILES, C], F16, kind="ExternalInput").ap()
    wkT = nc.dram_tensor("wkT", [P, 2, C], F16, kind="ExternalInput").ap()
    wvT = nc.dram_tensor("wvT", [P, 2, C], F16, kind="ExternalInput").ap()
    ident_d = nc.dram_tensor("ident", [P, P], F16, kind="ExternalInput").ap()
    oT = nc.dram_tensor("oT", [P, 2, N_FULL], F16, kind="ExternalOutput").ap()

    with tile.TileContext(nc) as tc:
        _build_kernel(tc, a_pm, wkT, wvT, ident_d, oT)
    nc.compile()
    return nc


def _build_kernel(tc, a_pm, wkT_d, wvT_d, ident_d, oT):
    nc = tc.nc

    with (
        tc.tile_pool(name="persist", bufs=1) as persist,
        tc.tile_pool(name="stage", bufs=3) as stage,
        tc.tile_pool(name="dram", bufs=1, space="DRAM") as dram,
        tc.tile_pool(name="psum_ga", bufs=1, space="PSUM") as psum_ga,
        tc.tile_pool(name="psum_gb", bufs=1, space="PSUM") as psum_gb,
        tc.tile_pool(name="psum_l", bufs=1, space="PSUM") as psum_l,
        tc.tile_pool(name="psum_big", bufs=5, space="PSUM") as psum_big,
    ):
        # ---- persistent SBUF state ----
        in_sb = [
            persist.tile([P, TPC, C], F16, tag=f"in{ch}", name=f"in{ch}")
            for ch in range(N_CHUNKS)
        ]
        qT_sb = [
            persist.tile([P, 2, TPC, P], F16, tag=f"qT{ch}", name=f"qT{ch}")
            for ch in range(N_CHUNKS)
        ]
        w_k = persist.tile([P, 2, C], F16, tag="w_k")
        w_v = persist.tile([P, 2, C], F16, tag="w_v")
        ident = persist.tile([P, P], F16, tag="ident")
        g16 = persist.tile([P, 2, C], F16, tag="g16")
        T16 = persist.tile([P, 2, C], F16, tag="T16")
        lgc = persist.tile([P, 2, HD], F32, tag="lgc")    # compact logits^T
        cT_own = persist.tile([P, 2, HD], F32, tag="cT_own")
        cT_sum = persist.tile([P, 2, HD], F32, tag="cT_sum")
        cT_peer = persist.tile([P, 2, HD], F32, tag="cT_peer")
        ctx16 = persist.tile([P, 2, P], F16, tag="ctx16")  # blockdiag, fp16

        b_in = dram.tile([P, 2, HD], F32, tag="b_in", name="b_in")
        b_out = dram.tile([P, 2, HD], F32, tag="b_out", name="b_out")

        # ---- input stream first (chunk 0 split for an earlier Gram start) ----
        nc.sync.dma_start(in_sb[0][:, ts(0, TPC // 2), :], a_pm[:, ts(0, TPC // 2), :])
        nc.sync.dma_start(
            in_sb[0][:, ds(TPC // 2, TPC // 2), :], a_pm[:, ds(TPC // 2, TPC // 2), :]
        )
        for ch in range(1, N_CHUNKS):
            nc.sync.dma_start(in_sb[ch][:], a_pm[:, ts(ch, TPC), :])
        nc.sync.dma_start(w_k[:], wkT_d)
        nc.sync.dma_start(w_v[:], wvT_d)
        nc.sync.dma_start(ident[:], ident_d)
        nc.vector.memset(ctx16[:], 0.0)

        # ---- phase 1: Gram G = A^T A; one accumulation region PER BANK ----
        ctx_last = {}  # engine -> last ctx-chain instruction (ordering fences)
        copy_first = {}

        def transpose_pair(ch, tp):
            """PE-transpose tiles (2*tp, 2*tp+1) of chunk ch into qT_sb."""
            pt = psum_big.tile([P, 2, 2, P], F32, tag="big", name=f"pt{ch}_{tp}")
            for i in range(2):
                for t2 in range(2):
                    nc.tensor.matmul(
                        pt[:, i, t2, :],
                        in_sb[ch][:, tp * 2 + t2, ts(i, P)],
                        ident[:], start=True, stop=True,
                    )
            dst = qT_sb[ch][:, :, ts(tp, 2), :]
            # same destination tile -> keep one engine per chunk (avoids
            # cross-engine write serialization), alternate by chunk parity
            eng = "v" if ch % 2 == 0 else "s"
            if eng == "v":
                cp = nc.vector.tensor_copy(dst, pt[:])
            else:
                cp = nc.scalar.copy(dst, pt[:])
            # keep the ctx->collective critical chain ahead of the copy
            # flood in each engine's in-order queue
            if eng not in copy_first:
                copy_first[eng] = True
                if eng in ctx_last:
                    add_dep_helper(
                        cp.ins, ctx_last[eng].ins, sync=False,
                        reason="qT copies wait for ctx chain",
                    )

        pga = psum_ga.tile([P, 2, C], F32, tag="ga", name="pga")  # G[0:128, :]
        pgb = psum_gb.tile([P, 2, C], F32, tag="gb", name="pgb")  # G[128:256, :]
        for ch in range(N_CHUNKS):
            for t in range(TPC):
                ti = ch * TPC + t
                tile_ap = in_sb[ch][:, t, :]
                nc.tensor.matmul(
                    pga[:, 0, :], tile_ap[:, ts(0, P)], tile_ap,
                    start=(ti == 0), stop=(ti == N_TILES - 1),
                )
                nc.tensor.matmul(
                    pgb[:, 0, :], tile_ap[:, ts(1, P)], tile_ap,
                    start=(ti == 0), stop=(ti == N_TILES - 1),
                )
        nc.vector.tensor_copy(g16[:, 0, :], pga[:, 0, :])
        nc.scalar.copy(g16[:, 1, :], pgb[:, 0, :])

        # ---- ctx_own: T = G @ Wk^T (G symmetric), logits = Wv @ T ----
        # pT blocks reuse the two Gram banks (same tag ring -> sequenced).
        pTA = psum_ga.tile([P, 2, C], F32, tag="ga", name="pTA")  # T[0:128, :]
        pTB = psum_gb.tile([P, 2, C], F32, tag="gb", name="pTB")  # T[128:256, :]
        for blkc, pT in ((0, pTA), (1, pTB)):
            for ci in range(2):
                nc.tensor.matmul(
                    pT[:, 0, :], g16[:, ci, ts(blkc, P)], w_k[:, ci, :],
                    start=(ci == 0), stop=(ci == 1),
                )
        nc.vector.tensor_copy(T16[:, 0, :], pTA[:, 0, :])
        nc.scalar.copy(T16[:, 1, :], pTB[:, 0, :])

        for g in range(2):
            pl = psum_l.tile([P, P], F32, tag="pl", name=f"pl{g}")
            for ci in range(2):
                nc.tensor.matmul(
                    pl[:], w_v[:, ci, ts(g, P)], T16[:, ci, ts(g, P)],
                    start=(ci == 0), stop=(ci == 1),
                )
            # extract the 4 diagonal head blocks -> compact [128, 32]
            for h in range(4):
                nc.vector.tensor_copy(
                    lgc[ds(h * HD, HD), g, :], pl[ds(h * HD, HD), ds(h * HD, HD)]
                )
            # batched softmax over d (free axis) for all 4 heads at once
            mx = stage.tile([P, 1], F32, tag="mx", name=f"mx{g}")
            nc.vector.tensor_reduce(
                mx[:], lgc[:, g, :], axis=mybir.AxisListType.X, op=mybir.AluOpType.max
            )
            nmx = stage.tile([P, 1], F32, tag="nmx", name=f"nmx{g}")
            nc.vector.tensor_scalar_mul(nmx[:], mx[:], -SCALE)
            sm = stage.tile([P, HD], F32, tag="sm", name=f"sm{g}")
            ssum = stage.tile([P, 1], F32, tag="ssum", name=f"ssum{g}")
            ctx_last["s"] = nc.scalar.activation(
                sm[:], lgc[:, g, :], mybir.ActivationFunctionType.Exp,
                bias=nmx[:], scale=SCALE, accum_out=ssum[:],
            )
            rs = stage.tile([P, 1], F32, tag="rs", name=f"rs{g}")
            nc.vector.reciprocal(rs[:], ssum[:])
            smn = stage.tile([P, HD], F32, tag="smn", name=f"smn{g}")
            nc.vector.tensor_scalar_mul(smn[:], sm[:], rs[:])
            # per-head 32x32 transpose: [32h+e, d] -> [32h+d, e]
            ctx_last["v"] = nc.vector.transpose(cT_own[:, g, :], smn[:])

        # ---- exchange ctx with the pair core (peer = pairsum - own).
        # Bounce DMAs ride the idle sync queue: gpsimd's wait lanes get
        # polluted by the transpose-phase copies and fire ~13us late.
        nc.sync.dma_start(b_in[:], cT_own[:])
        nc.gpsimd.collective_compute(
            "AllReduce",
            mybir.AluOpType.add,
            replica_groups=REPLICA_GROUPS,
            ins=[b_in.opt()],
            outs=[b_out.opt()],
        )
        nc.sync.dma_start(cT_sum[:], b_out[:])

        # ---- overlap the collective (~26us): PE transposes the input ----
        for ch in range(N_CHUNKS):
            for tp in range(TPC // 2):
                transpose_pair(ch, tp)

        # peer = pairsum - own, subtracted/cast directly into the fp16
        # blockdiag slots (one fused op per head block keeps the
        # post-collective critical path as short as possible)
        for g in range(2):
            for h in range(4):
                nc.vector.tensor_sub(
                    ctx16[ds(h * HD, HD), g, ds(h * HD, HD)],
                    cT_sum[ds(h * HD, HD), g, :],
                    cT_own[ds(h * HD, HD), g, :],
                )

        # ---- out^T = ctx_blk (stationary) @ qT spans, fp16 staged, DMA out.
        # One staging tile + one DMA per matmul: the two copy engines and the
        # two DMA trigger queues (sync HWDGE, gpsimd SWDGE) run concurrently.
        for g in range(2):
            for q in range(N_TILES // SPAN):
                ch, sp = divmod(q, TPC // SPAN)
                po = psum_big.tile([P, SPAN * P], F32, tag="big", name=f"po{g}_{q}")
                nc.tensor.matmul(
                    po[:], ctx16[:, g, :], qT_sb[ch][:, g, ts(sp, SPAN), :],
                    start=True, stop=True,
                )
                st = stage.tile(
                    [P, SPAN * P], F16, tag="st", bufs=16, name=f"st{g}_{q}"
                )
                if q % 2 == 0:
                    nc.vector.tensor_copy(st[:], po[:])
                    nc.sync.dma_start(oT[:, g, ts(q, SPAN * P)], st[:])
                else:
                    nc.scalar.copy(st[:], po[:])
                    nc.gpsimd.dma_start(oT[:, g, ts(q, SPAN * P)], st[:])


# ---------------------------------------------------------------------------
# Host-side wrapper
# ---------------------------------------------------------------------------

_NC_CACHE = {}


def _get_module(**kw):
    key = tuple(sorted(kw.items()))
    if key not in _NC_CACHE:
        _NC_CACHE[key] = build_module(**kw)
    return _NC_CACHE[key]


def make_in_maps(rgb, x, Wkv_rgb, Wkv_x, n_cores=8):
    """Per-core inputs. Core 2b+s owns stream s (0=rgb, 1=x) of batch b."""
    eye = np.eye(P, dtype=np.float16)
    in_maps = []
    for core in range(n_cores):
        b, s = divmod(core, 2)
        A = (rgb if s == 0 else x)[b]
        W = Wkv_rgb if s == 0 else Wkv_x
        a16 = A.astype(np.float16)
        a_pm = np.ascontiguousarray(a16.reshape(N_TILES, P, C).transpose(1, 0, 2))
        WkT = W[:C].T.reshape(2, P, C).transpose(1, 0, 2)   # [p, ci, col]
        WvT = W[C:].T.reshape(2, P, C).transpose(1, 0, 2)
        in_maps.append(
            {
                "a_pm": a_pm,
                "wkT": np.ascontiguousarray(WkT.astype(np.float16)),
                "wvT": np.ascontiguousarray(WvT.astype(np.float16)),
                "ident": eye,
            }
        )
    return in_maps


def assemble(results):
    out_rgb = np.empty((B_FULL, N_FULL, C), dtype=np.float32)
    out_x = np.empty_like(out_rgb)
    for core, res in enumerate(results):
        b, s = divmod(core, 2)
        o = res["oT"].transpose(2, 1, 0).reshape(N_FULL, C).astype(np.float32)
        (out_rgb if s == 0 else out_x)[b] = o
    return out_rgb, out_x


def kernel(rgb, x, Wkv_rgb, Wkv_x, num_heads):
    rgb = np.asarray(rgb, dtype=np.float32)
    x = np.asarray(x, dtype=np.float32)
    Wkv_rgb = np.asarray(Wkv_rgb, dtype=np.float32)
    Wkv_x = np.asarray(Wkv_x, dtype=np.float32)
    assert int(num_heads) == H_FULL
    assert rgb.shape == (B_FULL, N_FULL, C) and x.shape == (B_FULL, N_FULL, C)

    nc = _get_module()
    in_maps = make_in_maps(rgb, x, Wkv_rgb, Wkv_x)
    res = run_bass_kernel_spmd(nc, in_maps, core_ids=list(range(8)))
    return assemble(res.results)


# revision 32
# speedup vs baseline: 1.0496x; 1.0032x over previous
"""Trainium2 Bass kernel for the CrossAttention (linear-attention style) module.

Math (per batch b, head h, stream s in {rgb, x}):
    K = A_s @ Wk_s^T, V = A_s @ Wv_s^T            (A_s = stream input [N, C])
    ctx_s = softmax(scale * K^T V, axis=rows)     # [32, 32] per head
    out_s = A_s @ blockdiag(ctx_{s'})             # s' = the OTHER stream

Key identity: K^T V = Wk (A^T A) Wv^T, so the big inputs only feed the Gram
matrix G = A^T A (one [256,256] per (batch, stream)); the rest is tiny.

Sharding: 8 cores = 4 batches x 2 streams.  Core 2b+s owns stream s of
batch b: it streams A_s once (fp16, partition-major layout prepared on
host), computes G via PSUM-accumulated matmuls, derives its own ctx_s
on-chip, then swaps ctx with its pair core through a tiny AllReduce
(peer = sum - own, so the SPMD program stays core-independent).  While the
collective is in flight the PE transposes the retained input tiles
(plain matmuls against identity - NOT transpose-mode, which is slow).
Finally out^T = blockdiag(ctx_peer) block-stationary @ A^T in just
64 N=512 matmuls, written fp16 transposed; the host untransposes.

Hardware notes baked in:
 - Each Gram accumulation region gets its OWN PSUM bank: a start=True
   matmul clears has_written BANK-WIDE, so interleaving two accumulation
   groups in one bank corrupts the other region's in-flight tile
   (measured: split banks are exact to 1e-3 absolute; shared bank loses
   ~half a tile per group start).
 - PSUM->SBUF cast copies run ~1 elem/cycle/lane on both DVE and ACT,
   ~780 ns per [128,512]; they are the real phase gate, so they alternate
   between the two engines everywhere.

Everything on the streamed path is fp16: PE runs at full rate and DMA
traffic halves vs fp32 (8 MB in + 8 MB out per core).  Verified by a host
simulation of the exact pipeline: rel err ~5.6e-3 (gate is 2e-2).
"""

import sys

if "/opt/trn_rl_repo" not in sys.path:
    sys.path.insert(0, "/opt/trn_rl_repo")

import numpy as np

import concourse.bass as bass
import concourse.mybir as mybir
import concourse.tile as tile
from concourse import bacc
from concourse.bass import ds, ts
from concourse.bass_utils import run_bass_kernel_spmd
from concourse.tile_rust import add_dep_helper

P = 128
C = 256
HD = 32
SCALE = HD ** -0.5
F16 = mybir.dt.float16
F32 = mybir.dt.float32

B_FULL = 4
N_FULL = 16384
H_FULL = 8

N_TILES = N_FULL // P          # 128
TPC = 16                       # tiles per chunk
N_CHUNKS = N_TILES // TPC      # 8
SPAN = 4                       # qT tiles per out-matmul -> N = 512

REPLICA_GROUPS = [[0, 1], [2, 3], [4, 5], [6, 7]]


def build_module(num_devices=8):
    nc = bacc.Bacc(
        "TRN2",
        target_bir_lowering=False,
        debug=False,
        enable_asserts=False,
        num_devices=num_devices,
    )
    a_pm = nc.dram_tensor("a_pm", [P, N_TILES, C], F16, kind="ExternalInput").ap()
    wkT = nc.dram_tensor("wkT", [P, 2, C], F16, kind="ExternalInput").ap()
    wvT = nc.dram_tensor("wvT", [P, 2, C], F16, kind="ExternalInput").ap()
    ident_d = nc.dram_tensor("ident", [P, P], F16, kind="ExternalInput").ap()
    oT = nc.dram_tensor("oT", [P, 2, N_FULL], F16, kind="ExternalOutput").ap()

    with tile.TileContext(nc) as tc:
        _build_kernel(tc, a_pm, wkT, wvT, ident_d, oT)
    nc.compile()
    return nc


def _build_kernel(tc, a_pm, wkT_d, wvT_d, ident_d, oT):
    nc = tc.nc

    with (
        tc.tile_pool(name="persist", bufs=1) as persist,
        tc.tile_pool(name="stage", bufs=3) as stage,
        tc.tile_pool(name="dram", bufs=1, space="DRAM") as dram,
        tc.tile_pool(name="psum_ga", bufs=1, space="PSUM") as psum_ga,
        tc.tile_pool(name="psum_gb", bufs=1, space="PSUM") as psum_gb,
        tc.tile_pool(name="psum_l", bufs=1, space="PSUM") as psum_l,
        tc.tile_pool(name="psum_big", bufs=5, space="PSUM") as psum_big,
    ):
        # ---- persistent SBUF state ----
        in_sb = [
            persist.tile([P, TPC, C], F16, tag=f"in{ch}", name=f"in{ch}")
            for ch in range(N_CHUNKS)
        ]
        qT_sb = [
            persist.tile([P, 2, TPC, P], F16, tag=f"qT{ch}", name=f"qT{ch}")
            for ch in range(N_CHUNKS)
        ]
        w_k = persist.tile([P, 2, C], F16, tag="w_k")
        w_v = persist.tile([P, 2, C], F16, tag="w_v")
        ident = persist.tile([P, P], F16, tag="ident")
        g16 = persist.tile([P, 2, C], F16, tag="g16")
        T16 = persist.tile([P, 2, C], F16, tag="T16")
        lgc = persist.tile([P, 2, HD], F32, tag="lgc")    # compact logits^T
        cT_own = persist.tile([P, 2, HD], F32, tag="cT_own")
        cT_sum = persist.tile([P, 2, HD], F32, tag="cT_sum")
        cT_peer = persist.tile([P, 2, HD], F32, tag="cT_peer")
        ctx16 = persist.tile([P, 2, P], F16, tag="ctx16")  # blockdiag, fp16

        b_in = dram.tile([P, 2, HD], F32, tag="b_in", name="b_in")
        b_out = dram.tile([P, 2, HD], F32, tag="b_out", name="b_out")

        # ---- input stream first (chunk 0 split for an earlier Gram start) ----
        for j in range(4):
            nc.sync.dma_start(
                in_sb[0][:, ts(j, TPC // 4), :], a_pm[:, ts(j, TPC // 4), :]
            )
        for ch in range(1, N_CHUNKS):
            nc.sync.dma_start(in_sb[ch][:], a_pm[:, ts(ch, TPC), :])
        nc.sync.dma_start(w_k[:], wkT_d)
        nc.sync.dma_start(w_v[:], wvT_d)
        nc.sync.dma_start(ident[:], ident_d)
        nc.vector.memset(ctx16[:], 0.0)

        # ---- phase 1: Gram G = A^T A; one accumulation region PER BANK ----
        ctx_last = {}  # engine -> last ctx-chain instruction (ordering fences)
        copy_first = {}

        def transpose_pair(ch, tp):
            """PE-transpose tiles (2*tp, 2*tp+1) of chunk ch into qT_sb."""
            pt = psum_big.tile([P, 2, 2, P], F32, tag="big", name=f"pt{ch}_{tp}")
            for i in range(2):
                for t2 in range(2):
                    nc.tensor.matmul(
                        pt[:, i, t2, :],
                        in_sb[ch][:, tp * 2 + t2, ts(i, P)],
                        ident[:], start=True, stop=True,
                    )
            dst = qT_sb[ch][:, :, ts(tp, 2), :]
            # same destination tile -> keep one engine per chunk (avoids
            # cross-engine write serialization), alternate by chunk parity
            eng = "v" if ch % 2 == 0 else "s"
            if eng == "v":
                cp = nc.vector.tensor_copy(dst, pt[:])
            else:
                cp = nc.scalar.copy(dst, pt[:])
            # keep the ctx->collective critical chain ahead of the copy
            # flood in each engine's in-order queue
            if eng not in copy_first:
                copy_first[eng] = True
                if eng in ctx_last:
                    add_dep_helper(
                        cp.ins, ctx_last[eng].ins, sync=False,
                        reason="qT copies wait for ctx chain",
                    )

        pga = psum_ga.tile([P, 2, C], F32, tag="ga", name="pga")  # G[0:128, :]
        pgb = psum_gb.tile([P, 2, C], F32, tag="gb", name="pgb")  # G[128:256, :]
        for ch in range(N_CHUNKS):
            for t in range(TPC):
                ti = ch * TPC + t
                tile_ap = in_sb[ch][:, t, :]
                nc.tensor.matmul(
                    pga[:, 0, :], tile_ap[:, ts(0, P)], tile_ap,
                    start=(ti == 0), stop=(ti == N_TILES - 1),
                )
                nc.tensor.matmul(
                    pgb[:, 0, :], tile_ap[:, ts(1, P)], tile_ap,
                    start=(ti == 0), stop=(ti == N_TILES - 1),
                )
        nc.vector.tensor_copy(g16[:, 0, :], pga[:, 0, :])
        nc.scalar.copy(g16[:, 1, :], pgb[:, 0, :])

        # ---- ctx_own: T = G @ Wk^T (G symmetric), logits = Wv @ T ----
        # pT blocks reuse the two Gram banks (same tag ring -> sequenced).
        pTA = psum_ga.tile([P, 2, C], F32, tag="ga", name="pTA")  # T[0:128, :]
        pTB = psum_gb.tile([P, 2, C], F32, tag="gb", name="pTB")  # T[128:256, :]
        for blkc, pT in ((0, pTA), (1, pTB)):
            for ci in range(2):
                nc.tensor.matmul(
                    pT[:, 0, :], g16[:, ci, ts(blkc, P)], w_k[:, ci, :],
                    start=(ci == 0), stop=(ci == 1),
                )
        nc.vector.tensor_copy(T16[:, 0, :], pTA[:, 0, :])
        nc.scalar.copy(T16[:, 1, :], pTB[:, 0, :])

        for g in range(2):
            pl = psum_l.tile([P, P], F32, tag="pl", name=f"pl{g}")
            for ci in range(2):
                nc.tensor.matmul(
                    pl[:], w_v[:, ci, ts(g, P)], T16[:, ci, ts(g, P)],
                    start=(ci == 0), stop=(ci == 1),
                )
            # extract the 4 diagonal head blocks -> compact [128, 32]
            for h in range(4):
                nc.vector.tensor_copy(
                    lgc[ds(h * HD, HD), g, :], pl[ds(h * HD, HD), ds(h * HD, HD)]
                )
            # batched softmax over d (free axis) for all 4 heads at once
            mx = stage.tile([P, 1], F32, tag="mx", name=f"mx{g}")
            nc.vector.tensor_reduce(
                mx[:], lgc[:, g, :], axis=mybir.AxisListType.X, op=mybir.AluOpType.max
            )
            nmx = stage.tile([P, 1], F32, tag="nmx", name=f"nmx{g}")
            nc.vector.tensor_scalar_mul(nmx[:], mx[:], -SCALE)
            sm = stage.tile([P, HD], F32, tag="sm", name=f"sm{g}")
            ssum = stage.tile([P, 1], F32, tag="ssum", name=f"ssum{g}")
            ctx_last["s"] = nc.scalar.activation(
                sm[:], lgc[:, g, :], mybir.ActivationFunctionType.Exp,
                bias=nmx[:], scale=SCALE, accum_out=ssum[:],
            )
            rs = stage.tile([P, 1], F32, tag="rs", name=f"rs{g}")
            nc.vector.reciprocal(rs[:], ssum[:])
            smn = stage.tile([P, HD], F32, tag="smn", name=f"smn{g}")
            nc.vector.tensor_scalar_mul(smn[:], sm[:], rs[:])
            # per-head 32x32 transpose: [32h+e, d] -> [32h+d, e]
            ctx_last["v"] = nc.vector.transpose(cT_own[:, g, :], smn[:])

        # ---- exchange ctx with the pair core (peer = pairsum - own).
        # Bounce DMAs ride the idle sync queue: gpsimd's wait lanes get
        # polluted by the transpose-phase copies and fire ~13us late.
        nc.sync.dma_start(b_in[:], cT_own[:])
        nc.gpsimd.collective_compute(
            "AllReduce",
            mybir.AluOpType.add,
            replica_groups=REPLICA_GROUPS,
            ins=[b_in.opt()],
            outs=[b_out.opt()],
        )
        nc.sync.dma_start(cT_sum[:], b_out[:])

        # ---- overlap the collective (~26us): PE transposes the input ----
        for ch in range(N_CHUNKS):
            for tp in range(TPC // 2):
                transpose_pair(ch, tp)

        # keep the PE array busy across the collective delivery gap so the
        # out matmuls start at full clock (an ~8us idle would re-throttle
        # HAM to 1.2 GHz); harmless scratch matmuls, sized to end early
        for w in range(14):
            fl = psum_big.tile([P, 2, 2, P], F32, tag="big", name=f"warm{w}")
            nc.tensor.matmul(
                fl[:], in_sb[7][:, 14, ts(0, P)],
                in_sb[7][:, ds(14, 2), :], start=True, stop=True,
            )

        # peer = pairsum - own, subtracted/cast directly into the fp16
        # blockdiag slots, split DVE/gpsimd (all-SBUF operands) with the
        # g=0 half first so the first out matmul unblocks as early as
        # possible
        for g in range(2):
            for h in range(4):
                dst = ctx16[ds(h * HD, HD), g, ds(h * HD, HD)]
                s_sum = cT_sum[ds(h * HD, HD), g, :]
                s_own = cT_own[ds(h * HD, HD), g, :]
                eng = nc.vector if h % 2 == 0 else nc.gpsimd
                eng.tensor_sub(dst, s_sum, s_own)

        # ---- out^T = ctx_blk (stationary) @ qT spans, fp16 staged, DMA out.
        # One staging tile + one DMA per matmul: the two copy engines and the
        # two DMA trigger queues (sync HWDGE, gpsimd SWDGE) run concurrently.
        for g in range(2):
            for q in range(N_TILES // SPAN):
                ch, sp = divmod(q, TPC // SPAN)
                po = psum_big.tile([P, SPAN * P], F32, tag="big", name=f"po{g}_{q}")
                nc.tensor.matmul(
                    po[:], ctx16[:, g, :], qT_sb[ch][:, g, ts(sp, SPAN), :],
                    start=True, stop=True,
                )
                st = stage.tile(
                    [P, SPAN * P], F16, tag="st", bufs=16, name=f"st{g}_{q}"
                )
                if q % 2 == 0:
                    nc.vector.tensor_copy(st[:], po[:])
                    nc.sync.dma_start(oT[:, g, ts(q, SPAN * P)], st[:])
                else:
                    nc.scalar.copy(st[:], po[:])
                    nc.gpsimd.dma_start(oT[:, g, ts(q, SPAN * P)], st[:])


# ---------------------------------------------------------------------------
# Host-side wrapper
# ---------------------------------------------------------------------------

_NC_CACHE = {}


def _get_module(**kw):
    key = tuple(sorted(kw.items()))
    if key not in _NC_CACHE:
        _NC_CACHE[key] = build_module(**kw)
    return _NC_CACHE[key]


def make_in_maps(rgb, x, Wkv_rgb, Wkv_x, n_cores=8):
    """Per-core inputs. Core 2b+s owns stream s (0=rgb, 1=x) of batch b."""
    eye = np.eye(P, dtype=np.float16)
    in_maps = []
    for core in range(n_cores):
        b, s = divmod(core, 2)
        A = (rgb if s == 0 else x)[b]
        W = Wkv_rgb if s == 0 else Wkv_x
        a16 = A.astype(np.float16)
        a_pm = np.ascontiguousarray(a16.reshape(N_TILES, P, C).transpose(1, 0, 2))
        WkT = W[:C].T.reshape(2, P, C).transpose(1, 0, 2)   # [p, ci, col]
        WvT = W[C:].T.reshape(2, P, C).transpose(1, 0, 2)
        in_maps.append(
            {
                "a_pm": a_pm,
                "wkT": np.ascontiguousarray(WkT.astype(np.float16)),
                "wvT": np.ascontiguousarray(WvT.astype(np.float16)),
                "ident": eye,
            }
        )
    return in_maps


def assemble(results):
    out_rgb = np.empty((B_FULL, N_FULL, C), dtype=np.float32)
    out_x = np.empty_like(out_rgb)
    for core, res in enumerate(results):
        b, s = divmod(core, 2)
        o = res["oT"].transpose(2, 1, 0).reshape(N_FULL, C).astype(np.float32)
        (out_rgb if s == 0 else out_x)[b] = o
    return out_rgb, out_x


def kernel(rgb, x, Wkv_rgb, Wkv_x, num_heads):
    rgb = np.asarray(rgb, dtype=np.float32)
    x = np.asarray(x, dtype=np.float32)
    Wkv_rgb = np.asarray(Wkv_rgb, dtype=np.float32)
    Wkv_x = np.asarray(Wkv_x, dtype=np.float32)
    assert int(num_heads) == H_FULL
    assert rgb.shape == (B_FULL, N_FULL, C) and x.shape == (B_FULL, N_FULL, C)

    nc = _get_module()
    in_maps = make_in_maps(rgb, x, Wkv_rgb, Wkv_x)
    res = run_bass_kernel_spmd(nc, in_maps, core_ids=list(range(8)))
    return assemble(res.results)


# revision 35
# speedup vs baseline: 1.0720x; 1.0213x over previous
"""Trainium2 Bass kernel for the CrossAttention (linear-attention style) module.

Math (per batch b, head h, stream s in {rgb, x}):
    K = A_s @ Wk_s^T, V = A_s @ Wv_s^T            (A_s = stream input [N, C])
    ctx_s = softmax(scale * K^T V, axis=rows)     # [32, 32] per head
    out_s = A_s @ blockdiag(ctx_{s'})             # s' = the OTHER stream

Key identity: K^T V = Wk (A^T A) Wv^T, so the big inputs only feed the Gram
matrix G = A^T A (one [256,256] per (batch, stream)); the rest is tiny.

Sharding: 8 cores = 4 batches x 2 streams.  Core 2b+s owns stream s of
batch b: it streams A_s once (fp16, partition-major layout prepared on
host), computes G via PSUM-accumulated matmuls, derives its own ctx_s
on-chip, then swaps ctx with its pair core through a tiny AllReduce
(peer = sum - own, so the SPMD program stays core-independent).  While the
collective is in flight the PE transposes the retained input tiles
(plain matmuls against identity - NOT transpose-mode, which is slow).
Finally out^T = blockdiag(ctx_peer) block-stationary @ A^T in just
64 N=512 matmuls, written fp16 transposed; the host untransposes.

Hardware notes baked in:
 - Each Gram accumulation region gets its OWN PSUM bank: a start=True
   matmul clears has_written BANK-WIDE, so interleaving two accumulation
   groups in one bank corrupts the other region's in-flight tile
   (measured: split banks are exact to 1e-3 absolute; shared bank loses
   ~half a tile per group start).
 - PSUM->SBUF cast copies run ~1 elem/cycle/lane on both DVE and ACT,
   ~780 ns per [128,512]; they are the real phase gate, so they alternate
   between the two engines everywhere.

Everything on the streamed path is fp16: PE runs at full rate and DMA
traffic halves vs fp32 (8 MB in + 8 MB out per core).  Verified by a host
simulation of the exact pipeline: rel err ~5.6e-3 (gate is 2e-2).
"""

import sys

if "/opt/trn_rl_repo" not in sys.path:
    sys.path.insert(0, "/opt/trn_rl_repo")

import numpy as np

import concourse.bass as bass
import concourse.mybir as mybir
import concourse.tile as tile
from concourse import bacc
from concourse.bass import ds, ts
from concourse.bass_utils import run_bass_kernel_spmd
from concourse.tile_rust import add_dep_helper

P = 128
C = 256
HD = 32
SCALE = HD ** -0.5
F16 = mybir.dt.float16
F32 = mybir.dt.float32

B_FULL = 4
N_FULL = 16384
H_FULL = 8

N_TILES = N_FULL // P          # 128
TPC = 16                       # tiles per chunk
N_CHUNKS = N_TILES // TPC      # 8
SPAN = 4                       # qT tiles per out-matmul -> N = 512

REPLICA_GROUPS = [[0, 1], [2, 3], [4, 5], [6, 7]]


def build_module(num_devices=8):
    nc = bacc.Bacc(
        "TRN2",
        target_bir_lowering=False,
        debug=False,
        enable_asserts=False,
        num_devices=num_devices,
    )
    a_pm = nc.dram_tensor("a_pm", [P, N_TILES, C], F16, kind="ExternalInput").ap()
    wkT = nc.dram_tensor("wkT", [P, 2, C], F16, kind="ExternalInput").ap()
    wvT = nc.dram_tensor("wvT", [P, 2, C], F16, kind="ExternalInput").ap()
    ident_d = nc.dram_tensor("ident", [P, P], F16, kind="ExternalInput").ap()
    oT = nc.dram_tensor("oT", [P, 2, N_FULL], F16, kind="ExternalOutput").ap()

    with tile.TileContext(nc) as tc:
        _build_kernel(tc, a_pm, wkT, wvT, ident_d, oT)
    nc.compile()
    return nc


def _build_kernel(tc, a_pm, wkT_d, wvT_d, ident_d, oT):
    nc = tc.nc

    with (
        tc.tile_pool(name="persist", bufs=1) as persist,
        tc.tile_pool(name="stage", bufs=3) as stage,
        tc.tile_pool(name="dram", bufs=1, space="DRAM") as dram,
        tc.tile_pool(name="psum_ga", bufs=1, space="PSUM") as psum_ga,
        tc.tile_pool(name="psum_gb", bufs=1, space="PSUM") as psum_gb,
        tc.tile_pool(name="psum_l", bufs=1, space="PSUM") as psum_l,
        tc.tile_pool(name="psum_big", bufs=5, space="PSUM") as psum_big,
    ):
        # ---- persistent SBUF state ----
        in_sb = [
            persist.tile([P, TPC, C], F16, tag=f"in{ch}", name=f"in{ch}")
            for ch in range(N_CHUNKS)
        ]
        qT_sb = [
            persist.tile([P, 2, TPC, P], F16, tag=f"qT{ch}", name=f"qT{ch}")
            for ch in range(N_CHUNKS)
        ]
        w_k = persist.tile([P, 2, C], F16, tag="w_k")
        w_v = persist.tile([P, 2, C], F16, tag="w_v")
        ident = persist.tile([P, P], F16, tag="ident")
        g16 = persist.tile([P, 2, C], F16, tag="g16")
        T16 = persist.tile([P, 2, C], F16, tag="T16")
        lgc = persist.tile([P, 2, HD], F32, tag="lgc")    # compact logits^T
        cT_own = persist.tile([P, 2, HD], F32, tag="cT_own")
        cT_sum = persist.tile([P, 2, HD], F32, tag="cT_sum")
        cT_peer = persist.tile([P, 2, HD], F32, tag="cT_peer")
        ctx16 = persist.tile([P, 2, P], F16, tag="ctx16")  # blockdiag, fp16

        b_in = dram.tile([P, 2, HD], F32, tag="b_in", name="b_in")
        b_out = dram.tile([P, 2, HD], F32, tag="b_out", name="b_out")

        # ---- input stream first (chunk 0 split for an earlier Gram start) ----
        for j in range(4):
            nc.sync.dma_start(
                in_sb[0][:, ts(j, TPC // 4), :], a_pm[:, ts(j, TPC // 4), :]
            )
        for ch in range(1, N_CHUNKS):
            nc.sync.dma_start(in_sb[ch][:], a_pm[:, ts(ch, TPC), :])
        nc.sync.dma_start(w_k[:], wkT_d)
        nc.sync.dma_start(w_v[:], wvT_d)
        nc.sync.dma_start(ident[:], ident_d)
        nc.vector.memset(ctx16[:], 0.0)

        # ---- phase 1: Gram G = A^T A; one accumulation region PER BANK ----
        ctx_last = {}  # engine -> last ctx-chain instruction (ordering fences)
        copy_first = {}

        def transpose_pair(ch, tp):
            """PE-transpose tiles (2*tp, 2*tp+1) of chunk ch into qT_sb."""
            pt = psum_big.tile([P, 2, 2, P], F32, tag="big", name=f"pt{ch}_{tp}")
            for i in range(2):
                for t2 in range(2):
                    nc.tensor.matmul(
                        pt[:, i, t2, :],
                        in_sb[ch][:, tp * 2 + t2, ts(i, P)],
                        ident[:], start=True, stop=True,
                    )
            dst = qT_sb[ch][:, :, ts(tp, 2), :]
            # same destination tile -> keep one engine per chunk (avoids
            # cross-engine write serialization), alternate by chunk parity
            eng = "v" if ch % 2 == 0 else "s"
            if eng == "v":
                cp = nc.vector.tensor_copy(dst, pt[:])
            else:
                cp = nc.scalar.copy(dst, pt[:])
            # keep the ctx->collective critical chain ahead of the copy
            # flood in each engine's in-order queue
            if eng not in copy_first:
                copy_first[eng] = True
                if eng in ctx_last:
                    add_dep_helper(
                        cp.ins, ctx_last[eng].ins, sync=False,
                        reason="qT copies wait for ctx chain",
                    )

        pga = psum_ga.tile([P, 2, C], F32, tag="ga", name="pga")  # G[0:128, :]
        pgb = psum_gb.tile([P, 2, C], F32, tag="gb", name="pgb")  # G[128:256, :]
        for ch in range(N_CHUNKS):
            for t in range(TPC):
                ti = ch * TPC + t
                tile_ap = in_sb[ch][:, t, :]
                nc.tensor.matmul(
                    pga[:, 0, :], tile_ap[:, ts(0, P)], tile_ap,
                    start=(ti == 0), stop=(ti == N_TILES - 1),
                )
                nc.tensor.matmul(
                    pgb[:, 0, :], tile_ap[:, ts(1, P)], tile_ap,
                    start=(ti == 0), stop=(ti == N_TILES - 1),
                )
        nc.vector.tensor_copy(g16[:, 0, :], pga[:, 0, :])
        nc.scalar.copy(g16[:, 1, :], pgb[:, 0, :])

        # ---- ctx_own: T = G @ Wk^T (G symmetric), logits = Wv @ T ----
        # pT blocks reuse the two Gram banks (same tag ring -> sequenced).
        pTA = psum_ga.tile([P, 2, C], F32, tag="ga", name="pTA")  # T[0:128, :]
        pTB = psum_gb.tile([P, 2, C], F32, tag="gb", name="pTB")  # T[128:256, :]
        for blkc, pT in ((0, pTA), (1, pTB)):
            for ci in range(2):
                nc.tensor.matmul(
                    pT[:, 0, :], g16[:, ci, ts(blkc, P)], w_k[:, ci, :],
                    start=(ci == 0), stop=(ci == 1),
                )
        nc.vector.tensor_copy(T16[:, 0, :], pTA[:, 0, :])
        nc.scalar.copy(T16[:, 1, :], pTB[:, 0, :])

        for g in range(2):
            pl = psum_l.tile([P, P], F32, tag="pl", name=f"pl{g}")
            for ci in range(2):
                nc.tensor.matmul(
                    pl[:], w_v[:, ci, ts(g, P)], T16[:, ci, ts(g, P)],
                    start=(ci == 0), stop=(ci == 1),
                )
            # extract the 4 diagonal head blocks -> compact [128, 32]
            for h in range(4):
                nc.vector.tensor_copy(
                    lgc[ds(h * HD, HD), g, :], pl[ds(h * HD, HD), ds(h * HD, HD)]
                )
            # batched softmax over d (free axis) for all 4 heads at once
            mx = stage.tile([P, 1], F32, tag="mx", name=f"mx{g}")
            nc.vector.tensor_reduce(
                mx[:], lgc[:, g, :], axis=mybir.AxisListType.X, op=mybir.AluOpType.max
            )
            nmx = stage.tile([P, 1], F32, tag="nmx", name=f"nmx{g}")
            nc.vector.tensor_scalar_mul(nmx[:], mx[:], -SCALE)
            sm = stage.tile([P, HD], F32, tag="sm", name=f"sm{g}")
            ssum = stage.tile([P, 1], F32, tag="ssum", name=f"ssum{g}")
            ctx_last["s"] = nc.scalar.activation(
                sm[:], lgc[:, g, :], mybir.ActivationFunctionType.Exp,
                bias=nmx[:], scale=SCALE, accum_out=ssum[:],
            )
            rs = stage.tile([P, 1], F32, tag="rs", name=f"rs{g}")
            nc.vector.reciprocal(rs[:], ssum[:])
            smn = stage.tile([P, HD], F32, tag="smn", name=f"smn{g}")
            nc.vector.tensor_scalar_mul(smn[:], sm[:], rs[:])
            # per-head 32x32 transpose: [32h+e, d] -> [32h+d, e]
            ctx_last["v"] = nc.vector.transpose(cT_own[:, g, :], smn[:])

        # ---- exchange ctx with the pair core (peer = pairsum - own).
        # Bounce DMAs ride the idle sync queue: gpsimd's wait lanes get
        # polluted by the transpose-phase copies and fire ~13us late.
        nc.sync.dma_start(b_in[:], cT_own[:])
        nc.gpsimd.collective_compute(
            "AllReduce",
            mybir.AluOpType.add,
            replica_groups=REPLICA_GROUPS,
            ins=[b_in.opt()],
            outs=[b_out.opt()],
        )
        nc.sync.dma_start(cT_sum[:], b_out[:])

        # ---- overlap the collective (~26us): PE transposes the input ----
        for ch in range(N_CHUNKS):
            for tp in range(TPC // 2):
                transpose_pair(ch, tp)

        # keep the PE array busy across the collective delivery gap so the
        # out matmuls start at full clock (an ~8us idle would re-throttle
        # HAM to 1.2 GHz); harmless scratch matmuls, sized to end early
        for w in range(14):
            fl = psum_big.tile([P, 2, 2, P], F32, tag="big", name=f"warm{w}")
            nc.tensor.matmul(
                fl[:], in_sb[7][:, 14, ts(0, P)],
                in_sb[7][:, ds(14, 2), :], start=True, stop=True,
            )

        # peer = pairsum - own, subtracted/cast directly into the fp16
        # blockdiag slots, split DVE/gpsimd (all-SBUF operands) with the
        # g=0 half first so the first out matmul unblocks as early as
        # possible
        for g in range(2):
            for h in range(4):
                dst = ctx16[ds(h * HD, HD), g, ds(h * HD, HD)]
                s_sum = cT_sum[ds(h * HD, HD), g, :]
                s_own = cT_own[ds(h * HD, HD), g, :]
                eng = nc.vector if h % 2 == 0 else nc.gpsimd
                eng.tensor_sub(dst, s_sum, s_own)

        # ---- out^T = ctx_blk (stationary) @ qT spans, fp16 staged, DMA out.
        # One staging tile + one DMA per matmul: the two copy engines and the
        # two DMA trigger queues (sync HWDGE, gpsimd SWDGE) run concurrently.
        for g in range(2):
            for q in range(N_TILES // SPAN):
                ch, sp = divmod(q, TPC // SPAN)
                po = psum_big.tile([P, SPAN * P], F32, tag="big", name=f"po{g}_{q}")
                nc.tensor.matmul(
                    po[:], ctx16[:, g, :], qT_sb[ch][:, g, ts(sp, SPAN), :],
                    start=True, stop=True,
                )
                st = stage.tile(
                    [P, SPAN * P], F16, tag="st", bufs=16, name=f"st{g}_{q}"
                )
                if q % 2 == 0:
                    nc.vector.tensor_copy(st[:], po[:])
                    nc.sync.dma_start(oT[:, g, ts(q, SPAN * P)], st[:])
                else:
                    nc.scalar.copy(st[:], po[:])
                    nc.gpsimd.dma_start(oT[:, g, ts(q, SPAN * P)], st[:])


# ---------------------------------------------------------------------------
# Host-side wrapper
# ---------------------------------------------------------------------------

_NC_CACHE = {}


def _get_module(**kw):
    key = tuple(sorted(kw.items()))
    if key not in _NC_CACHE:
        _NC_CACHE[key] = build_module(**kw)
    return _NC_CACHE[key]


def make_in_maps(rgb, x, Wkv_rgb, Wkv_x, n_cores=8):
    """Per-core inputs. Core 2b+s owns stream s (0=rgb, 1=x) of batch b."""
    eye = np.eye(P, dtype=np.float16)
    in_maps = []
    for core in range(n_cores):
        b, s = divmod(core, 2)
        A = (rgb if s == 0 else x)[b]
        W = Wkv_rgb if s == 0 else Wkv_x
        a16 = A.astype(np.float16)
        a_pm = np.ascontiguousarray(a16.reshape(N_TILES, P, C).transpose(1, 0, 2))
        WkT = W[:C].T.reshape(2, P, C).transpose(1, 0, 2)   # [p, ci, col]
        WvT = W[C:].T.reshape(2, P, C).transpose(1, 0, 2)
        in_maps.append(
            {
                "a_pm": a_pm,
                "wkT": np.ascontiguousarray(WkT.astype(np.float16)),
                "wvT": np.ascontiguousarray(WvT.astype(np.float16)),
                "ident": eye,
            }
        )
    return in_maps


def assemble(results):
    out_rgb = np.empty((B_FULL, N_FULL, C), dtype=np.float32)
    out_x = np.empty_like(out_rgb)
    for core, res in enumerate(results):
        b, s = divmod(core, 2)
        o = res["oT"].transpose(2, 1, 0).reshape(N_FULL, C).astype(np.float32)
        (out_rgb if s == 0 else out_x)[b] = o
    return out_rgb, out_x


def kernel(rgb, x, Wkv_rgb, Wkv_x, num_heads):
    rgb = np.asarray(rgb, dtype=np.float32)
    x = np.asarray(x, dtype=np.float32)
    Wkv_rgb = np.asarray(Wkv_rgb, dtype=np.float32)
    Wkv_x = np.asarray(Wkv_x, dtype=np.float32)
    assert int(num_heads) == H_FULL
    assert rgb.shape == (B_FULL, N_FULL, C) and x.shape == (B_FULL, N_FULL, C)

    nc = _get_module()
    in_maps = make_in_maps(rgb, x, Wkv_rgb, Wkv_x)
    res = run_bass_kernel_spmd(nc, in_maps, core_ids=list(range(8)))
    return assemble(res.results)
